# revision 3
# baseline (speedup 1.0000x reference)
"""Trainium2 Bass kernel for batched filtfilt band-pass filtering (tensorpac-style).

Math: filtfilt with FIR taps b == one convolution of the odd-extended input with
autocorr(b) on the interior (see baseline docstring).  out[n] = sum_d A[d] ext[P+n+d].

This version cuts tensor-engine work three ways relative to the all-fp16 banded
matmul baseline:
  1. Tail truncation: each band's autocorr support is truncated where the tail
     energy is negligible (rel err ~1e-3), trimming Toeplitz block count.
  2. fp8 DoubleRow pairs: outer (low-energy) Toeplitz blocks are computed in
     fp8e4 with perf_mode=DoubleRow -- TWO 128-row blocks per matmul at ~half
     the per-block cost.  Per-band scaling (2^mk) keeps fp8 operands in range;
     the host unscales during reassembly (power-of-2, exact in fp16).
  3. Band pairing: bands whose truncated half-support D <= 32 are packed two
     per matmul (64 output rows each) using 32/96-shifted copies of ext^T, so a
     K=128 matmul covers both bands' diagonals -- 2 MMs per group for 2 bands.

Device mapping per core (sequence-parallel over 8 cores) otherwise follows the
baseline: (3072,128) position-major ext slice in SBUF-native [p,h,b] layout,
PSUM accumulation per band in one 4-bank [128,2048] tile, DVE/ACT split drains
(cast to fp16), out-DMAs alternating over the ACT/gpsimd HWDGE rings.
"""

import os

import numpy as np
import ml_dtypes

import concourse.mybir as mybir
from concourse import bacc
from concourse.tile import TileContext
from concourse.bass_utils import run_bass_kernel_spmd
from concourse.ap import AP

F32 = mybir.dt.float32
F16 = mybir.dt.float16
F8 = mybir.dt.float8e4
E4M3 = ml_dtypes.float8_e4m3

B = 128          # batch
L = 16384        # sequence length
P = 512          # padlen (= TAPS - 1)
NB = 20          # bands
N_CORES = 8
LC = L // N_CORES            # 2048 output positions per core
GROUPS = LC // 512           # 4 groups of 512 positions
EXT_ROWS = LC + 2 * P        # 3072 ext rows per core (halo included)
H_E = EXT_ROWS // 128        # 24 aligned 128-row blocks
H_SH = (EXT_ROWS - 128) // 128  # 23 shifted blocks (64/32/96 + 128h + p)
N_WARM = 14
USE_FP8 = os.environ.get("KERNEL_NO_FP8", "") == ""

LAST_RESULT = None
_program_cache: dict = {}

EPS8 = 0.036 * np.sqrt(2.0)
BAND_CAP = float(os.environ.get("KERNEL_BAND_CAP", "1.2e-2"))
LAM = 3e4


def _acorr(kernels):
    """Per-band (t, autocorr) in fp64."""
    out = []
    for k in range(kernels.shape[0]):
        nz = np.nonzero(kernels[k])[0]
        t = int(nz[-1]) + 1 if nz.size else 1
        b = kernels[k][:t].astype(np.float64)
        out.append((t, np.correlate(b, b, mode="full")))
    return out


def _plan(kernels):
    """Choose per-band config: solo (D,s,Q,fp16 blocks,fp8 pairs) or pair."""
    acorrs = _acorr(kernels)
    energies = np.array([(A ** 2).sum() for _, A in acorrs])
    esh = energies / energies.sum()

    # pair-eligible: D<=32 drop err small
    pair_drop = {}
    for k, (t, A) in enumerate(acorrs):
        dv = np.abs(np.arange(2 * t - 1) - (t - 1))
        D = min(32, t - 1)
        pair_drop[k] = ((A[dv > D] ** 2).sum() / (A ** 2).sum(), D)
    elig = [k for k in range(NB) if pair_drop[k][0] < 1.0e-2 ** 2]
    npairs = len(elig) // 2
    paired = {}
    for i in range(npairs):
        a, b = elig[2 * i], elig[2 * i + 1]
        paired[a] = b

    plan = {}
    for i, (a, b) in enumerate(paired.items()):
        plan[a] = dict(kind="pairA", partner=b, sec=i, mk=0)
        plan[b] = dict(kind="pairB", partner=a, sec=i, mk=0)

    for k in range(NB):
        if k in plan:
            continue
        t, A = acorrs[k]
        tot = (A ** 2).sum()
        ds = np.arange(-(t - 1), t)
        dabs = np.abs(ds)
        drop = np.array([(A[dabs > D] ** 2).sum() / tot for D in range(t)])
        Dset = {t - 1}
        for eps in (1e-4, 3e-4, 1e-3, 2e-3, 3e-3, 5e-3):
            ok = np.where(drop <= eps ** 2)[0]
            if ok.size:
                Dset.add(int(ok[0]))
        best = None
        for D in sorted(Dset):
            s = 64 * ((D + 63) // 64) if D > 0 else 64
            Q = -(-(s + D + 1 + 127) // 128)
            A2 = (A ** 2)[dabs <= D]
            base = ds[dabs <= D] + s
            lo = base // 128
            rem = base % 128
            for nf in range(1, Q + 1):
                for q0 in range(0, Q - nf + 1):
                    q1 = q0 + nf
                    n8 = Q - nf
                    if n8 > 0 and not USE_FP8:
                        continue
                    in_lo = (lo >= q0) & (lo < q1)
                    in_hi = (lo + 1 >= q0) & (lo + 1 < q1)
                    f16frac = in_lo * (128 - rem) / 128.0 + in_hi * rem / 128.0
                    f8 = float((A2 * (1.0 - f16frac)).sum() / tot)
                    err2 = (EPS8 ** 2) * f8 + drop[D]
                    if np.sqrt(err2) > BAND_CAP:
                        continue
                    cost = nf + 1.0 * ((n8 + 1) // 2)
                    obj = cost + LAM * err2 * esh[k]
                    if best is None or obj < best[0]:
                        fp8b = [q for q in range(Q) if not (q0 <= q < q1)]
                        best = (obj, dict(
                            kind="solo", D=int(D), s=int(s), Q=int(Q),
                            fp16=list(range(q0, q1)), fp8=fp8b))
        cfg = best[1]
        # fp8 scale: lhs8 = A * 2^(mk-4); rhs8 = ext * 16
        t, A = acorrs[k]
        mk = 0
        if cfg["fp8"]:
            # max |A| over fp8-owned coeffs ~ max over blocks outside fp16 win
            s, D = cfg["s"], cfg["D"]
            kkv = np.arange(128)[:, None]
            rrv = np.arange(128)[None, :]
            mx = 0.0
            for q in cfg["fp8"]:
                d = 128 * q - s + kkv - rrv
                m = np.abs(d) <= D
                if m.any():
                    vals = np.abs(A[np.clip(d[m] + t - 1, 0, 2 * t - 2)])
                    mx = max(mx, float(vals.max()) if vals.size else 0.0)
            if mx > 0:
                mk4 = int(np.floor(np.log2(200.0 / mx)))
                mk = mk4 + 4
            a0 = float(np.abs(A).max())
            while a0 * 2.0 ** mk > 28000.0:
                mk -= 1
            # fp16 output range: |y| <~ 12*sqrt(band energy) incl. edge spikes
            ymax = 12.0 * float(np.sqrt((A ** 2).sum()))
            while ymax * 2.0 ** mk > 30000.0:
                mk -= 1
        cfg["mk"] = mk
        plan[k] = cfg
    return plan, acorrs


def _band_order(plan):
    """First: fp16-only band with s%128==0 (uses E, streamed early).
    E64-dependent bands next; fp8/E8_64 and pair bands later; small last."""
    solos = [k for k in range(NB) if plan[k]["kind"] == "solo"]
    pairsA = [k for k in range(NB) if plan[k]["kind"] == "pairA"]

    def cost(k):
        c = plan[k]
        return len(c["fp16"]) + 1.0 * ((len(c["fp8"]) + 1) // 2)

    e_solo = [k for k in solos if plan[k]["s"] % 128 == 0 and not plan[k]["fp8"]]
    e64_solo = [k for k in solos if plan[k]["s"] % 128 == 64 and not plan[k]["fp8"]]
    fp8_bands = [k for k in solos if plan[k]["fp8"]]
    # fp8 bands: E8 (s%128==0) before E8_64 ones
    fp8_e = sorted([k for k in fp8_bands if plan[k]["s"] % 128 == 0],
                   key=cost, reverse=True)
    fp8_e64 = sorted([k for k in fp8_bands if plan[k]["s"] % 128 == 64],
                     key=cost, reverse=True)

    e_solo.sort(key=cost)
    e64_solo.sort(key=cost)
    order = []
    order.append(("solo", e_solo[0]))          # first: cheap, E-aligned
    rest_e = e_solo[1:]
    rest_e64 = list(e64_solo)
    # second..fourth: more E-aligned fp16 work while E64/E8 still stream in
    for src in (rest_e, rest_e, rest_e):
        if src:
            order.append(("solo", src.pop(0)))
    mids = fp8_e + fp8_e64
    pair_list = [("pair", k) for k in pairsA]
    tail_pool = [("solo", k) for k in rest_e64 + rest_e]
    mids_i = [("solo", k) for k in mids]
    # interleave: fp8-heavy bands spread among fp16 bands and pairs
    merged = []
    pools = [mids_i, tail_pool, pair_list]
    while any(pools):
        for p in pools:
            if p:
                merged.append(p.pop(0))
    order += merged
    # ensure last is a small fp16 solo (short tail)
    for i in range(len(order) - 1, -1, -1):
        kind, k = order[i]
        if kind == "solo" and not plan[k]["fp8"] and len(plan[k]["fp16"]) <= 2:
            order.append(order.pop(i))
            break
    return order


def _build_consts(kernels, plan, acorrs, order):
    """lhs16 stacked blocks, lhs8 stacked pairs, pair-band lhsT, offsets."""
    kk = np.arange(128)[:, None]
    rr = np.arange(128)[None, :]
    l16_blocks = []
    l8_pairs = []
    off16 = {}
    off8 = {}
    for kind, k in order:
        if kind == "pair":
            kb = plan[k]["partner"]
            rr64 = np.arange(64)[None, :]
            blocks = []
            for t_half in range(2):
                blk = np.zeros((128, 128), np.float64)
                for half, band in ((0, k), (1, kb)):
                    t, A = acorrs[band]
                    d = kk - 32 - rr64
                    D = min(32, t - 1)
                    v = np.where(np.abs(d) <= D,
                                 A[np.clip(d + t - 1, 0, 2 * t - 2)], 0.0)
                    blk[:, half * 64:(half + 1) * 64] = v
                blocks.append(blk)
            off16[("pair", k)] = len(l16_blocks)
            l16_blocks += [b.astype(np.float16) for b in blocks]
            continue
        cfg = plan[k]
        t, A = acorrs[k]
        D, s, mk = cfg["D"], cfg["s"], cfg["mk"]

        def block(q, scale):
            d = 128 * q - s + kk - rr
            v = np.where(np.abs(d) <= D,
                         A[np.clip(d + t - 1, 0, 2 * t - 2)], 0.0)
            return v * scale

        off16[("solo", k)] = len(l16_blocks)
        for q in cfg["fp16"]:
            l16_blocks.append(block(q, 2.0 ** mk).astype(np.float16))
        fp8 = cfg["fp8"]
        prs = [(fp8[i], fp8[i + 1]) if i + 1 < len(fp8) else (fp8[i], None)
               for i in range(0, len(fp8), 2)]
        off8[("solo", k)] = len(l8_pairs)
        for qa, qb in prs:
            pa = np.clip(block(qa, 2.0 ** (mk - 4)), -240, 240).astype(np.float32).astype(E4M3)
            if qb is None:
                pb = np.zeros((128, 128), E4M3)
            else:
                pb = np.clip(block(qb, 2.0 ** (mk - 4)), -240, 240).astype(np.float32).astype(E4M3)
            l8_pairs.append(np.stack([pa, pb], axis=1))  # [kk, 2, r]
        cfg["pairs"] = prs
    lhs16 = (np.stack(l16_blocks, axis=1) if l16_blocks
             else np.zeros((128, 1, 128), np.float16))  # [kk, blk, r]
    lhs8 = (np.stack(l8_pairs, axis=1) if l8_pairs
            else np.zeros((128, 1, 2, 128), E4M3))      # [kk, pair, 2, r]
    return np.ascontiguousarray(lhs16), np.ascontiguousarray(lhs8), off16, off8


def _plan_key(plan, order):
    items = []
    for kind, k in order:
        c = plan[k]
        if c["kind"] == "solo":
            items.append((kind, k, c["D"], c["s"], c["Q"],
                          tuple(c["fp16"]), tuple(c["fp8"]), c["mk"]))
        else:
            items.append((kind, k, c["partner"]))
    return tuple(items)


def _build_program(plan, order, off16, off8, n16, n8):
    key = (_plan_key(plan, order), n16, n8)
    if key in _program_cache:
        return _program_cache[key]

    nc = bacc.Bacc("TRN2", target_bir_lowering=False, debug=False,
                   num_devices=N_CORES)
    ext_in = nc.declare_dram_parameter("ext", [128, H_E, B], F16, isOutput=False)
    ext64_in = nc.declare_dram_parameter("ext64", [128, H_SH, B], F16, isOutput=False)
    ext32_in = nc.declare_dram_parameter("ext32", [128, H_SH, B], F16, isOutput=False)
    ext96_in = nc.declare_dram_parameter("ext96", [128, H_SH, B], F16, isOutput=False)
    ext8_in = nc.declare_dram_parameter("ext8", [128, H_E, B], F8, isOutput=False)
    ext864_in = nc.declare_dram_parameter("ext864", [128, H_SH, B], F8, isOutput=False)
    l16_in = nc.declare_dram_parameter("lhs16", [128, n16, 128], F16, isOutput=False)
    l8_in = nc.declare_dram_parameter("lhs8", [128, n8, 2, 128], F8, isOutput=False)
    n_solo = sum(1 for kind, _ in order if kind == "solo")
    n_pair = sum(1 for kind, _ in order if kind == "pair")
    out_t = nc.declare_dram_parameter("out", [n_solo, 128, GROUPS * 512], F16,
                                      isOutput=True)
    out_p = (nc.declare_dram_parameter("outp", [max(n_pair, 1), 128, GROUPS * 1024],
                                       F16, isOutput=True))

    with TileContext(nc) as tc:
        with (
            tc.tile_pool(name="consts", bufs=1) as cpool,
            tc.tile_pool(name="psum", bufs=4, space="PSUM") as ppool,
            tc.tile_pool(name="ostage", bufs=4) as opool,
        ):
            E = cpool.tile([128, H_E * 128], F16)
            E64 = cpool.tile([128, H_SH * 128], F16)
            E32 = cpool.tile([128, H_SH * 128], F16)
            E96 = cpool.tile([128, H_SH * 128], F16)
            E8 = cpool.tile([128, H_E * 128], F8)
            E8_64 = cpool.tile([128, H_SH * 128], F8)
            Lw16a = cpool.tile([128, max(n16, 1) * 128], F16, name="Lw16a")
            Lw16b = cpool.tile([128, max(n16, 1) * 128], F16, name="Lw16b")
            Lw8 = cpool.tile([128, n8 * 256], F8)
            warm = cpool.tile([128, 256], F16)
            wps = ppool.tile([128, 1024], F32, name="ps")

            nc.any.memset(warm[:], 0.0)
            for _ in range(N_WARM):
                nc.tensor.matmul(wps[:, 0:256], warm[:, :128], warm[:],
                                 start=True, stop=True)

            # input streams: sync ring carries E + E64 + lhs (band-ordered);
            # scalar ring carries the later-needed fp8/pair ext variants.
            e_flat = ext_in[:].rearrange("p h b -> p (h b)")
            c1 = 10 * 128
            nc.sync.dma_start(out=E[:, 0:c1], in_=e_flat[:, 0:c1])

            # constants: first 3 bands' lhs16 individually (early MMs), then
            # the rest in bulk; E tail, E64 and lhs8 follow on sync.
            nsplit = 0
            for kind, k in order[:4]:
                if kind != "solo" or plan[k]["pairs"]:
                    break
                nsplit = off16[("solo", k)] + len(plan[k]["fp16"])
            nc.sync.dma_start(
                out=Lw16a[:, 0:nsplit * 128].rearrange("kk (i r) -> kk i r", r=128),
                in_=l16_in[:, 0:nsplit, :])
            nc.sync.dma_start(out=E[:, c1:], in_=e_flat[:, c1:])
            nc.sync.dma_start(out=E64[:], in_=ext64_in[:].rearrange("p h b -> p (h b)"))
            nc.sync.dma_start(
                out=Lw8[:].rearrange("kk (i r) -> kk i r", r=256),
                in_=l8_in[:].rearrange("kk i two r -> kk i (two r)"))
            nc.scalar.dma_start(
                out=Lw16b[:, nsplit * 128:].rearrange("kk (i r) -> kk i r", r=128),
                in_=l16_in[:, nsplit:, :])
            nc.scalar.dma_start(out=E8[:], in_=ext8_in[:].rearrange("p h b -> p (h b)"))
            nc.scalar.dma_start(out=E8_64[:], in_=ext864_in[:].rearrange("p h b -> p (h b)"))
            nc.scalar.dma_start(out=E32[:], in_=ext32_in[:].rearrange("p h b -> p (h b)"))
            nc.scalar.dma_start(out=E96[:], in_=ext96_in[:].rearrange("p h b -> p (h b)"))

            solo_idx = {}
            pair_idx = {}
            si = 0
            for kind, k in order:
                if kind == "solo":
                    solo_idx[k] = si
                    si += 1
                else:
                    pair_idx[k] = plan[k]["sec"]

            first_band = order[0][1]
            out_ring = [nc.scalar, nc.gpsimd]
            ring_i = 0
            pending = [None]  # (ob, solo_idx of first half)

            def flush_pending():
                if pending[0] is None:
                    return
                ob0, si0 = pending[0]
                eng = out_ring[ring_i % 2]
                eng.dma_start(out=out_t[si0], in_=ob0[:, 0:2048])
                pending[0] = None

            for oi, (kind, k) in enumerate(order):
                last = oi == len(order) - 1
                if kind == "pair":
                    o = off16[("pair", k)]
                    ob = opool.tile([128, GROUPS * 1024], F16, name="ob")
                    flush_pending()
                    for g in range(GROUPS):
                        ps = ppool.tile([128, 1024], F32, name="ps")
                        for t_half in range(2):
                            src = E96 if t_half == 0 else E32
                            h0 = 4 * g + 3 + t_half
                            nc.tensor.matmul(
                                ps[:, t_half * 512:(t_half + 1) * 512],
                                Lw16b[:, (o + t_half) * 128:(o + t_half + 1) * 128],
                                src[:, h0 * 128:h0 * 128 + 512],
                                start=True, stop=True)
                        if g < 2:
                            nc.vector.tensor_copy(
                                ob[:, g * 1024:(g + 1) * 1024], ps[:])
                        else:
                            nc.scalar.copy(
                                ob[:, g * 1024:(g + 1) * 1024], ps[:])
                    eng = out_ring[ring_i % 2]
                    ring_i += 1
                    eng.dma_start(out=out_p[pair_idx[k]], in_=ob[:])
                    continue

                cfg = plan[k]
                s, mk = cfg["s"], cfg["mk"]
                use64 = (s % 128) == 64
                src16 = E64 if use64 else E
                h_base16 = (P - 64 - s) // 128 if use64 else (P - s) // 128
                src8 = E8_64 if use64 else E8
                if last:
                    flush_pending()
                    ob = opool.tile([128, GROUPS * 512], F16, name="ob")
                    obslice = ob[:, 0:2048]
                elif pending[0] is None:
                    ob = opool.tile([128, GROUPS * 1024], F16, name="ob")
                    obslice = ob[:, 0:2048]
                else:
                    ob, si0 = pending[0]
                    obslice = ob[:, 2048:4096]
                ps01 = ppool.tile([128, 1024], F32, name="ps")
                ps23 = ppool.tile([128, 1024], F32, name="ps")
                o = off16[("solo", k)]
                o8 = off8[("solo", k)]
                nf = len(cfg["fp16"])
                npr = len(cfg["pairs"])

                def psw(g):
                    t = ps01 if g < 2 else ps23
                    return t[:, (g % 2) * 512:(g % 2 + 1) * 512]

                def mm_fp16(qi, g, st, sp):
                    q = cfg["fp16"][qi]
                    h0 = 4 * g + h_base16 + q
                    lw = Lw16a if o + qi < nsplit else Lw16b
                    nc.tensor.matmul(
                        psw(g),
                        lw[:, (o + qi) * 128:(o + qi + 1) * 128],
                        src16[:, h0 * 128:h0 * 128 + 512],
                        start=st, stop=sp)

                def mm_fp8(pi, g, st, sp):
                    qa, qb = cfg["pairs"][pi]
                    h0 = 4 * g + h_base16 + qa
                    dq = (qb - qa) if qb is not None else 1
                    a = src8[:]
                    rhs = AP(a.tensor, a.offset + h0 * 128,
                             [[H_SH * 128 if use64 else H_E * 128, 128],
                              [dq * 128, 2], [128, 4], [1, 128]])
                    lhs = Lw8[:, (o8 + pi) * 256:(o8 + pi + 1) * 256].rearrange(
                        "kk (two r) -> kk two r", two=2)
                    nc.tensor.matmul(
                        psw(g), lhs, rhs,
                        start=st, stop=sp,
                        perf_mode=mybir.MatmulPerfMode.DoubleRow)

                n_steps = nf + npr
                if k == first_band:
                    for g in range(GROUPS):
                        for qi in range(nf):
                            mm_fp16(qi, g, qi == 0, qi == n_steps - 1)
                        for pi in range(npr):
                            mm_fp8(pi, g, nf + pi == 0, nf + pi == n_steps - 1)
                else:
                    for qi in range(nf):
                        for g in range(GROUPS):
                            mm_fp16(qi, g, qi == 0, qi == n_steps - 1)
                    for pi in range(npr):
                        for g in range(GROUPS):
                            mm_fp8(pi, g, nf + pi == 0, nf + pi == n_steps - 1)

                nc.vector.tensor_copy(obslice[:, 0:1024], ps01[:])
                nc.scalar.copy(obslice[:, 1024:2048], ps23[:])
                if last:
                    nc.sync.dma_start(out=out_t[solo_idx[k], :, 0:1024],
                                      in_=ob[:, 0:1024])
                    nc.gpsimd.dma_start(out=out_t[solo_idx[k], :, 1024:2048],
                                        in_=ob[:, 1024:2048])
                elif pending[0] is None:
                    pending[0] = (ob, solo_idx[k])
                else:
                    ob0, si0 = pending[0]
                    assert solo_idx[k] == si0 + 1
                    eng = out_ring[ring_i % 2]
                    ring_i += 1
                    eng.dma_start(
                        out=out_t[si0:si0 + 2].rearrange("i p c -> p i c"),
                        in_=ob0[:].rearrange("p (i c) -> p i c", i=2))
                    pending[0] = None

    nc.compile()
    _program_cache[key] = (nc, solo_idx, pair_idx)
    return _program_cache[key]


def _maybe_register_trace_hook():
    try:
        import sys
        import types

        import antenv

        if getattr(antenv, "axon_hooks", None) is not None:
            return
        from trn_agent_boot.trn_boot import _ntff_profile_via_ctypes

        hooks = types.ModuleType("antenv.axon_hooks")
        hook = _ntff_profile_via_ctypes("/opt/axon/libaxon_pjrt.so")
        hooks.get_axon_ntff_profile_hook = lambda: hook
        hooks.set_axon_ntff_profile_hook = lambda h: None
        antenv.axon_hooks = hooks
        sys.modules["antenv.axon_hooks"] = hooks
    except Exception:
        pass


def kernel(x: np.ndarray, kernels: np.ndarray, padlen) -> np.ndarray:
    global LAST_RESULT
    x = np.asarray(x, dtype=np.float32)
    kernels = np.asarray(kernels, dtype=np.float32)
    assert x.shape == (B, 1, L) and kernels.shape[0] == NB
    assert int(padlen) == P

    plan, acorrs = _plan(kernels)
    order = _band_order(plan)
    lhs16, lhs8, off16, off8 = _build_consts(kernels, plan, acorrs, order)
    n16, n8 = lhs16.shape[1], lhs8.shape[1]
    nc, solo_idx, pair_idx = _build_program(plan, order, off16, off8, n16, n8)

    x2d = x[:, 0, :]
    left = 2.0 * x2d[:, :1] - x2d[:, 1:P + 1][:, ::-1]
    right = 2.0 * x2d[:, -1:] - x2d[:, -P - 1:-1][:, ::-1]
    ext = np.concatenate([left, x2d, right], axis=1)
    ext_t = ext.T.astype(np.float16)                      # (L+2P, B)
    ext8_t = (ext.T * 16.0).astype(np.float32).astype(E4M3)

    def pack(arr, base, nh):
        sl = arr[base: base + nh * 128]
        return np.ascontiguousarray(sl.reshape(nh, 128, -1).transpose(1, 0, 2))

    in_maps = []
    for c in range(N_CORES):
        b0 = c * LC
        in_maps.append({
            "ext": pack(ext_t, b0, H_E),
            "ext64": pack(ext_t, b0 + 64, H_SH),
            "ext32": pack(ext_t, b0 + 32, H_SH),
            "ext96": pack(ext_t, b0 + 96, H_SH),
            "ext8": pack(ext8_t, b0, H_E),
            "ext864": pack(ext8_t, b0 + 64, H_SH),
            "lhs16": lhs16, "lhs8": lhs8,
        })

    trace = bool(os.environ.get("KERNEL_TRACE"))
    if trace:
        _maybe_register_trace_hook()
    res = run_bass_kernel_spmd(nc, in_maps, list(range(N_CORES)), trace=trace)
    LAST_RESULT = res

    out = np.empty((B, 1, NB, L), np.float32)
    for c in range(N_CORES):
        dev = res.results[c]["out"].astype(np.float32)
        for k, si in solo_idx.items():
            mk = plan[k]["mk"]
            arr = dev[si].reshape(128, GROUPS, 4, B)      # [r, g, j, b]
            band = arr.transpose(3, 1, 2, 0).reshape(B, LC) * 2.0 ** (-mk)
            out[:, 0, k, c * LC:(c + 1) * LC] = band
        devp = res.results[c]["outp"].astype(np.float32)
        done = set()
        for k, sec in pair_idx.items():
            if k in done:
                continue
            kb = plan[k].get("partner")
            if plan[k]["kind"] == "pairB":
                k, kb = kb, k
            done.add(k)
            done.add(kb)
            arr = devp[sec].reshape(128, GROUPS, 2, 4, B)  # [r, g, t, j, b]
            a = arr[:64]                                   # [rr, g, t, j, b]
            bb = arr[64:]
            # position = 512g + 128j + 64t + rr
            bandA = a.transpose(4, 1, 3, 2, 0).reshape(B, LC)
            bandB = bb.transpose(4, 1, 3, 2, 0).reshape(B, LC)
            out[:, 0, k, c * LC:(c + 1) * LC] = bandA
            out[:, 0, kb, c * LC:(c + 1) * LC] = bandB
    return out


# revision 4
# speedup vs baseline: 1.1638x; 1.1638x over previous
"""Trainium2 Bass kernel for batched filtfilt band-pass filtering (tensorpac-style).

Math: filtfilt with FIR taps b == one convolution of the odd-extended input with
autocorr(b) on the interior (see baseline docstring).  out[n] = sum_d A[d] ext[P+n+d].

This version cuts tensor-engine work three ways relative to the all-fp16 banded
matmul baseline:
  1. Tail truncation: each band's autocorr support is truncated where the tail
     energy is negligible (rel err ~1e-3), trimming Toeplitz block count.
  2. fp8 DoubleRow pairs: outer (low-energy) Toeplitz blocks are computed in
     fp8e4 with perf_mode=DoubleRow -- TWO 128-row blocks per matmul at ~half
     the per-block cost.  Per-band scaling (2^mk) keeps fp8 operands in range;
     the host unscales during reassembly (power-of-2, exact in fp16).
  3. Band pairing: bands whose truncated half-support D <= 32 are packed two
     per matmul (64 output rows each) using 32/96-shifted copies of ext^T, so a
     K=128 matmul covers both bands' diagonals -- 2 MMs per group for 2 bands.

Device mapping per core (sequence-parallel over 8 cores) otherwise follows the
baseline: (3072,128) position-major ext slice in SBUF-native [p,h,b] layout,
PSUM accumulation per band in one 4-bank [128,2048] tile, DVE/ACT split drains
(cast to fp16), out-DMAs alternating over the ACT/gpsimd HWDGE rings.
"""

import os

import numpy as np
import ml_dtypes

import concourse.mybir as mybir
from concourse import bacc
from concourse.tile import TileContext
from concourse.bass_utils import run_bass_kernel_spmd
from concourse.ap import AP

F32 = mybir.dt.float32
F16 = mybir.dt.float16
F8 = mybir.dt.float8e4
E4M3 = ml_dtypes.float8_e4m3

B = 128          # batch
L = 16384        # sequence length
P = 512          # padlen (= TAPS - 1)
NB = 20          # bands
N_CORES = 8
LC = L // N_CORES            # 2048 output positions per core
GROUPS = LC // 512           # 4 groups of 512 positions
EXT_ROWS = LC + 2 * P        # 3072 ext rows per core (halo included)
H_E = EXT_ROWS // 128        # 24 aligned 128-row blocks
H_SH = (EXT_ROWS - 128) // 128  # 23 shifted blocks (64/32/96 + 128h + p)
N_WARM = 14
USE_FP8 = os.environ.get("KERNEL_NO_FP8", "") == ""

LAST_RESULT = None
_program_cache: dict = {}

EPS8 = 0.036 * np.sqrt(2.0)
BAND_CAP = float(os.environ.get("KERNEL_BAND_CAP", "1.2e-2"))
LAM = 3e4


def _acorr(kernels):
    """Per-band (t, autocorr) in fp64."""
    out = []
    for k in range(kernels.shape[0]):
        nz = np.nonzero(kernels[k])[0]
        t = int(nz[-1]) + 1 if nz.size else 1
        b = kernels[k][:t].astype(np.float64)
        out.append((t, np.correlate(b, b, mode="full")))
    return out


def _plan(kernels):
    """Choose per-band config: solo (D,s,Q,fp16 blocks,fp8 pairs) or pair."""
    acorrs = _acorr(kernels)
    energies = np.array([(A ** 2).sum() for _, A in acorrs])
    esh = energies / energies.sum()

    # pair-eligible: D<=32 drop err small
    pair_drop = {}
    for k, (t, A) in enumerate(acorrs):
        dv = np.abs(np.arange(2 * t - 1) - (t - 1))
        D = min(32, t - 1)
        pair_drop[k] = ((A[dv > D] ** 2).sum() / (A ** 2).sum(), D)
    elig = [k for k in range(NB) if pair_drop[k][0] < 1.0e-2 ** 2]
    npairs = len(elig) // 2
    paired = {}
    for i in range(npairs):
        a, b = elig[2 * i], elig[2 * i + 1]
        paired[a] = b

    plan = {}
    for i, (a, b) in enumerate(paired.items()):
        plan[a] = dict(kind="pairA", partner=b, sec=i, mk=0)
        plan[b] = dict(kind="pairB", partner=a, sec=i, mk=0)

    for k in range(NB):
        if k in plan:
            continue
        t, A = acorrs[k]
        tot = (A ** 2).sum()
        ds = np.arange(-(t - 1), t)
        dabs = np.abs(ds)
        drop = np.array([(A[dabs > D] ** 2).sum() / tot for D in range(t)])
        Dset = {t - 1}
        for eps in (1e-4, 3e-4, 1e-3, 2e-3, 3e-3, 5e-3):
            ok = np.where(drop <= eps ** 2)[0]
            if ok.size:
                Dset.add(int(ok[0]))
        best = None
        for D in sorted(Dset):
            s = 64 * ((D + 63) // 64) if D > 0 else 64
            Q = -(-(s + D + 1 + 127) // 128)
            A2 = (A ** 2)[dabs <= D]
            base = ds[dabs <= D] + s
            lo = base // 128
            rem = base % 128
            for nf in range(1, Q + 1):
                for q0 in range(0, Q - nf + 1):
                    q1 = q0 + nf
                    n8 = Q - nf
                    if n8 > 0 and not USE_FP8:
                        continue
                    in_lo = (lo >= q0) & (lo < q1)
                    in_hi = (lo + 1 >= q0) & (lo + 1 < q1)
                    f16frac = in_lo * (128 - rem) / 128.0 + in_hi * rem / 128.0
                    f8 = float((A2 * (1.0 - f16frac)).sum() / tot)
                    err2 = (EPS8 ** 2) * f8 + drop[D]
                    if np.sqrt(err2) > BAND_CAP:
                        continue
                    cost = nf + 1.0 * ((n8 + 1) // 2)
                    obj = cost + LAM * err2 * esh[k]
                    if best is None or obj < best[0]:
                        fp8b = [q for q in range(Q) if not (q0 <= q < q1)]
                        best = (obj, dict(
                            kind="solo", D=int(D), s=int(s), Q=int(Q),
                            fp16=list(range(q0, q1)), fp8=fp8b))
        cfg = best[1]
        # fp8 scale: lhs8 = A * 2^(mk-4); rhs8 = ext * 16
        t, A = acorrs[k]
        mk = 0
        if cfg["fp8"]:
            # max |A| over fp8-owned coeffs ~ max over blocks outside fp16 win
            s, D = cfg["s"], cfg["D"]
            kkv = np.arange(128)[:, None]
            rrv = np.arange(128)[None, :]
            mx = 0.0
            for q in cfg["fp8"]:
                d = 128 * q - s + kkv - rrv
                m = np.abs(d) <= D
                if m.any():
                    vals = np.abs(A[np.clip(d[m] + t - 1, 0, 2 * t - 2)])
                    mx = max(mx, float(vals.max()) if vals.size else 0.0)
            if mx > 0:
                mk4 = int(np.floor(np.log2(200.0 / mx)))
                mk = mk4 + 4
            a0 = float(np.abs(A).max())
            while a0 * 2.0 ** mk > 28000.0:
                mk -= 1
            # fp16 output range: |y| <~ 12*sqrt(band energy) incl. edge spikes
            ymax = 12.0 * float(np.sqrt((A ** 2).sum()))
            while ymax * 2.0 ** mk > 30000.0:
                mk -= 1
        cfg["mk"] = mk
        plan[k] = cfg
    return plan, acorrs


def _band_order(plan):
    """First: fp16-only band with s%128==0 (uses E, streamed early).
    E64-dependent bands next; fp8/E8_64 and pair bands later; small last."""
    solos = [k for k in range(NB) if plan[k]["kind"] == "solo"]
    pairsA = [k for k in range(NB) if plan[k]["kind"] == "pairA"]

    def cost(k):
        c = plan[k]
        return len(c["fp16"]) + 1.0 * ((len(c["fp8"]) + 1) // 2)

    e_solo = [k for k in solos if plan[k]["s"] % 128 == 0 and not plan[k]["fp8"]]
    e64_solo = [k for k in solos if plan[k]["s"] % 128 == 64 and not plan[k]["fp8"]]
    fp8_bands = [k for k in solos if plan[k]["fp8"]]
    # fp8 bands: E8 (s%128==0) before E8_64 ones
    fp8_e = sorted([k for k in fp8_bands if plan[k]["s"] % 128 == 0],
                   key=cost, reverse=True)
    fp8_e64 = sorted([k for k in fp8_bands if plan[k]["s"] % 128 == 64],
                     key=cost, reverse=True)

    e_solo.sort(key=cost)
    e64_solo.sort(key=cost)
    order = []
    order.append(("solo", e_solo[0]))          # first: cheap, E-aligned
    rest_e = e_solo[1:]
    rest_e64 = list(e64_solo)
    # second..fourth: more E-aligned fp16 work while E64/E8 still stream in
    for src in (rest_e, rest_e, rest_e):
        if src:
            order.append(("solo", src.pop(0)))
    mids = fp8_e + fp8_e64
    pair_list = [("pair", k) for k in pairsA]
    tail_pool = [("solo", k) for k in rest_e64 + rest_e]
    mids_i = [("solo", k) for k in mids]
    # interleave: fp8-heavy bands spread among fp16 bands and pairs
    merged = []
    pools = [mids_i, tail_pool, pair_list]
    while any(pools):
        for p in pools:
            if p:
                merged.append(p.pop(0))
    order += merged
    # ensure last is a small fp16 solo (short tail)
    for i in range(len(order) - 1, -1, -1):
        kind, k = order[i]
        if kind == "solo" and not plan[k]["fp8"] and len(plan[k]["fp16"]) <= 2:
            order.append(order.pop(i))
            break
    return order


def _build_consts(kernels, plan, acorrs, order):
    """lhs16 stacked blocks, lhs8 stacked pairs, pair-band lhsT, offsets."""
    kk = np.arange(128)[:, None]
    rr = np.arange(128)[None, :]
    l16_blocks = []
    l8_pairs = []
    off16 = {}
    off8 = {}
    for kind, k in order:
        if kind == "pair":
            kb = plan[k]["partner"]
            rr64 = np.arange(64)[None, :]
            blocks = []
            for t_half in range(2):
                blk = np.zeros((128, 128), np.float64)
                for half, band in ((0, k), (1, kb)):
                    t, A = acorrs[band]
                    d = kk - 32 - rr64
                    D = min(32, t - 1)
                    v = np.where(np.abs(d) <= D,
                                 A[np.clip(d + t - 1, 0, 2 * t - 2)], 0.0)
                    blk[:, half * 64:(half + 1) * 64] = v
                blocks.append(blk)
            off16[("pair", k)] = len(l16_blocks)
            l16_blocks += [b.astype(np.float16) for b in blocks]
            continue
        cfg = plan[k]
        t, A = acorrs[k]
        D, s, mk = cfg["D"], cfg["s"], cfg["mk"]

        def block(q, scale):
            d = 128 * q - s + kk - rr
            v = np.where(np.abs(d) <= D,
                         A[np.clip(d + t - 1, 0, 2 * t - 2)], 0.0)
            return v * scale

        off16[("solo", k)] = len(l16_blocks)
        for q in cfg["fp16"]:
            l16_blocks.append(block(q, 2.0 ** mk).astype(np.float16))
        fp8 = cfg["fp8"]
        prs = [(fp8[i], fp8[i + 1]) if i + 1 < len(fp8) else (fp8[i], None)
               for i in range(0, len(fp8), 2)]
        off8[("solo", k)] = len(l8_pairs)
        for qa, qb in prs:
            pa = np.clip(block(qa, 2.0 ** (mk - 4)), -240, 240).astype(np.float32).astype(E4M3)
            if qb is None:
                pb = np.zeros((128, 128), E4M3)
            else:
                pb = np.clip(block(qb, 2.0 ** (mk - 4)), -240, 240).astype(np.float32).astype(E4M3)
            l8_pairs.append(np.stack([pa, pb], axis=1))  # [kk, 2, r]
        cfg["pairs"] = prs
    lhs16 = (np.stack(l16_blocks, axis=1) if l16_blocks
             else np.zeros((128, 1, 128), np.float16))  # [kk, blk, r]
    lhs8 = (np.stack(l8_pairs, axis=1) if l8_pairs
            else np.zeros((128, 1, 2, 128), E4M3))      # [kk, pair, 2, r]
    return np.ascontiguousarray(lhs16), np.ascontiguousarray(lhs8), off16, off8


def _plan_key(plan, order):
    items = []
    for kind, k in order:
        c = plan[k]
        if c["kind"] == "solo":
            items.append((kind, k, c["D"], c["s"], c["Q"],
                          tuple(c["fp16"]), tuple(c["fp8"]), c["mk"]))
        else:
            items.append((kind, k, c["partner"]))
    return tuple(items)


def _build_program(plan, order, off16, off8, n16, n8):
    key = (_plan_key(plan, order), n16, n8)
    if key in _program_cache:
        return _program_cache[key]

    nc = bacc.Bacc("TRN2", target_bir_lowering=False, debug=False,
                   num_devices=N_CORES)
    ext_in = nc.declare_dram_parameter("ext", [128, H_E, B], F16, isOutput=False)
    ext64_in = nc.declare_dram_parameter("ext64", [128, H_SH, B], F16, isOutput=False)
    ext32_in = nc.declare_dram_parameter("ext32", [128, H_SH, B], F16, isOutput=False)
    ext96_in = nc.declare_dram_parameter("ext96", [128, H_SH, B], F16, isOutput=False)
    ext8_in = nc.declare_dram_parameter("ext8", [128, H_E, B], F8, isOutput=False)
    ext864_in = nc.declare_dram_parameter("ext864", [128, H_SH, B], F8, isOutput=False)
    l16_in = nc.declare_dram_parameter("lhs16", [128, n16, 128], F16, isOutput=False)
    l8_in = nc.declare_dram_parameter("lhs8", [128, n8, 2, 128], F8, isOutput=False)
    n_solo = sum(1 for kind, _ in order if kind == "solo")
    n_pair = sum(1 for kind, _ in order if kind == "pair")
    out_t = nc.declare_dram_parameter("out", [n_solo, 128, GROUPS * 512], F16,
                                      isOutput=True)
    out_p = (nc.declare_dram_parameter("outp", [max(n_pair, 1), 128, GROUPS * 1024],
                                       F16, isOutput=True))

    with TileContext(nc) as tc:
        with (
            tc.tile_pool(name="consts", bufs=1) as cpool,
            tc.tile_pool(name="psum", bufs=4, space="PSUM") as ppool,
            tc.tile_pool(name="ostage", bufs=4) as opool,
        ):
            E = cpool.tile([128, H_E * 128], F16)
            E64 = cpool.tile([128, H_SH * 128], F16)
            E32 = cpool.tile([128, H_SH * 128], F16)
            E96 = cpool.tile([128, H_SH * 128], F16)
            E8 = cpool.tile([128, H_E * 128], F8)
            E8_64 = cpool.tile([128, H_SH * 128], F8)
            Lw16 = cpool.tile([128, n16 * 128], F16)
            Lw8 = cpool.tile([128, n8 * 256], F8)
            warm = cpool.tile([128, 256], F16)
            wps = ppool.tile([128, 1024], F32, name="ps")

            nc.any.memset(warm[:], 0.0)
            for _ in range(N_WARM):
                nc.tensor.matmul(wps[:, 0:256], warm[:, :128], warm[:],
                                 start=True, stop=True)

            # input streams: sync ring carries E + E64 + lhs (band-ordered);
            # scalar ring carries the later-needed fp8/pair ext variants.
            e_flat = ext_in[:].rearrange("p h b -> p (h b)")
            c1 = 10 * 128
            nc.sync.dma_start(out=E[:, 0:c1], in_=e_flat[:, 0:c1])

            # constants: first 3 bands' lhs16 individually (early MMs), then
            # the rest in bulk; E tail, E64 and lhs8 follow on sync.
            nsplit = 0
            for kind, k in order[:3]:
                if kind != "solo":
                    break
                cfg = plan[k]
                o = off16[("solo", k)]
                nf = len(cfg["fp16"])
                if off8[("solo", k)] is not None and cfg["pairs"]:
                    break
                nc.sync.dma_start(
                    out=Lw16[:, o * 128:(o + nf) * 128].rearrange(
                        "kk (i r) -> kk i r", r=128),
                    in_=l16_in[:, o:o + nf, :])
                nsplit = o + nf
            nc.sync.dma_start(out=E[:, c1:], in_=e_flat[:, c1:])
            nc.sync.dma_start(out=E64[:], in_=ext64_in[:].rearrange("p h b -> p (h b)"))
            nc.sync.dma_start(
                out=Lw8[:].rearrange("kk (i r) -> kk i r", r=256),
                in_=l8_in[:].rearrange("kk i two r -> kk i (two r)"))
            nc.scalar.dma_start(
                out=Lw16[:, nsplit * 128:].rearrange("kk (i r) -> kk i r", r=128),
                in_=l16_in[:, nsplit:, :])
            nc.scalar.dma_start(out=E8[:], in_=ext8_in[:].rearrange("p h b -> p (h b)"))
            nc.scalar.dma_start(out=E8_64[:], in_=ext864_in[:].rearrange("p h b -> p (h b)"))
            nc.scalar.dma_start(out=E32[:], in_=ext32_in[:].rearrange("p h b -> p (h b)"))
            nc.scalar.dma_start(out=E96[:], in_=ext96_in[:].rearrange("p h b -> p (h b)"))

            solo_idx = {}
            pair_idx = {}
            si = 0
            for kind, k in order:
                if kind == "solo":
                    solo_idx[k] = si
                    si += 1
                else:
                    pair_idx[k] = plan[k]["sec"]

            first_band = order[0][1]
            out_ring = [nc.scalar, nc.gpsimd]
            ring_i = 0
            pending = [None]  # (ob, solo_idx of first half)

            def flush_pending():
                if pending[0] is None:
                    return
                ob0, si0 = pending[0]
                eng = out_ring[ring_i % 2]
                eng.dma_start(out=out_t[si0], in_=ob0[:, 0:2048])
                pending[0] = None

            for oi, (kind, k) in enumerate(order):
                last = oi == len(order) - 1
                if kind == "pair":
                    o = off16[("pair", k)]
                    ob = opool.tile([128, GROUPS * 1024], F16, name="ob")
                    flush_pending()
                    for g in range(GROUPS):
                        ps = ppool.tile([128, 1024], F32, name="ps")
                        for t_half in range(2):
                            src = E96 if t_half == 0 else E32
                            h0 = 4 * g + 3 + t_half
                            nc.tensor.matmul(
                                ps[:, t_half * 512:(t_half + 1) * 512],
                                Lw16[:, (o + t_half) * 128:(o + t_half + 1) * 128],
                                src[:, h0 * 128:h0 * 128 + 512],
                                start=True, stop=True)
                        if g < 2:
                            nc.vector.tensor_copy(
                                ob[:, g * 1024:(g + 1) * 1024], ps[:])
                        else:
                            nc.scalar.copy(
                                ob[:, g * 1024:(g + 1) * 1024], ps[:])
                    eng = out_ring[ring_i % 2]
                    ring_i += 1
                    eng.dma_start(out=out_p[pair_idx[k]], in_=ob[:])
                    continue

                cfg = plan[k]
                s, mk = cfg["s"], cfg["mk"]
                use64 = (s % 128) == 64
                src16 = E64 if use64 else E
                h_base16 = (P - 64 - s) // 128 if use64 else (P - s) // 128
                src8 = E8_64 if use64 else E8
                if last:
                    flush_pending()
                    ob = opool.tile([128, GROUPS * 512], F16, name="ob")
                    obslice = ob[:, 0:2048]
                elif pending[0] is None:
                    ob = opool.tile([128, GROUPS * 1024], F16, name="ob")
                    obslice = ob[:, 0:2048]
                else:
                    ob, si0 = pending[0]
                    obslice = ob[:, 2048:4096]
                ps01 = ppool.tile([128, 1024], F32, name="ps")
                ps23 = ppool.tile([128, 1024], F32, name="ps")
                o = off16[("solo", k)]
                o8 = off8[("solo", k)]
                nf = len(cfg["fp16"])
                npr = len(cfg["pairs"])

                def psw(g):
                    t = ps01 if g < 2 else ps23
                    return t[:, (g % 2) * 512:(g % 2 + 1) * 512]

                def mm_fp16(qi, g, st, sp):
                    q = cfg["fp16"][qi]
                    h0 = 4 * g + h_base16 + q
                    nc.tensor.matmul(
                        psw(g),
                        Lw16[:, (o + qi) * 128:(o + qi + 1) * 128],
                        src16[:, h0 * 128:h0 * 128 + 512],
                        start=st, stop=sp)

                def mm_fp8(pi, g, st, sp):
                    qa, qb = cfg["pairs"][pi]
                    h0 = 4 * g + h_base16 + qa
                    dq = (qb - qa) if qb is not None else 1
                    a = src8[:]
                    rhs = AP(a.tensor, a.offset + h0 * 128,
                             [[H_SH * 128 if use64 else H_E * 128, 128],
                              [dq * 128, 2], [128, 4], [1, 128]])
                    lhs = Lw8[:, (o8 + pi) * 256:(o8 + pi + 1) * 256].rearrange(
                        "kk (two r) -> kk two r", two=2)
                    nc.tensor.matmul(
                        psw(g), lhs, rhs,
                        start=st, stop=sp,
                        perf_mode=mybir.MatmulPerfMode.DoubleRow)

                n_steps = nf + npr
                if k == first_band:
                    for g in range(GROUPS):
                        for qi in range(nf):
                            mm_fp16(qi, g, qi == 0, qi == n_steps - 1)
                        for pi in range(npr):
                            mm_fp8(pi, g, nf + pi == 0, nf + pi == n_steps - 1)
                else:
                    for qi in range(nf):
                        for g in range(GROUPS):
                            mm_fp16(qi, g, qi == 0, qi == n_steps - 1)
                    for pi in range(npr):
                        for g in range(GROUPS):
                            mm_fp8(pi, g, nf + pi == 0, nf + pi == n_steps - 1)

                nc.vector.tensor_copy(obslice[:, 0:1024], ps01[:])
                nc.scalar.copy(obslice[:, 1024:2048], ps23[:])
                if last:
                    nc.sync.dma_start(out=out_t[solo_idx[k], :, 0:1024],
                                      in_=ob[:, 0:1024])
                    nc.gpsimd.dma_start(out=out_t[solo_idx[k], :, 1024:2048],
                                        in_=ob[:, 1024:2048])
                elif pending[0] is None:
                    pending[0] = (ob, solo_idx[k])
                else:
                    ob0, si0 = pending[0]
                    assert solo_idx[k] == si0 + 1
                    eng = out_ring[ring_i % 2]
                    ring_i += 1
                    eng.dma_start(
                        out=out_t[si0:si0 + 2].rearrange("i p c -> p i c"),
                        in_=ob0[:].rearrange("p (i c) -> p i c", i=2))
                    pending[0] = None

    nc.compile()
    _program_cache[key] = (nc, solo_idx, pair_idx)
    return _program_cache[key]


def _maybe_register_trace_hook():
    try:
        import sys
        import types

        import antenv

        if getattr(antenv, "axon_hooks", None) is not None:
            return
        from trn_agent_boot.trn_boot import _ntff_profile_via_ctypes

        hooks = types.ModuleType("antenv.axon_hooks")
        hook = _ntff_profile_via_ctypes("/opt/axon/libaxon_pjrt.so")
        hooks.get_axon_ntff_profile_hook = lambda: hook
        hooks.set_axon_ntff_profile_hook = lambda h: None
        antenv.axon_hooks = hooks
        sys.modules["antenv.axon_hooks"] = hooks
    except Exception:
        pass


def kernel(x: np.ndarray, kernels: np.ndarray, padlen) -> np.ndarray:
    global LAST_RESULT
    x = np.asarray(x, dtype=np.float32)
    kernels = np.asarray(kernels, dtype=np.float32)
    assert x.shape == (B, 1, L) and kernels.shape[0] == NB
    assert int(padlen) == P

    plan, acorrs = _plan(kernels)
    order = _band_order(plan)
    lhs16, lhs8, off16, off8 = _build_consts(kernels, plan, acorrs, order)
    n16, n8 = lhs16.shape[1], lhs8.shape[1]
    nc, solo_idx, pair_idx = _build_program(plan, order, off16, off8, n16, n8)

    x2d = x[:, 0, :]
    left = 2.0 * x2d[:, :1] - x2d[:, 1:P + 1][:, ::-1]
    right = 2.0 * x2d[:, -1:] - x2d[:, -P - 1:-1][:, ::-1]
    ext = np.concatenate([left, x2d, right], axis=1)
    ext_t = ext.T.astype(np.float16)                      # (L+2P, B)
    ext8_t = (ext.T * 16.0).astype(np.float32).astype(E4M3)

    def pack(arr, base, nh):
        sl = arr[base: base + nh * 128]
        return np.ascontiguousarray(sl.reshape(nh, 128, -1).transpose(1, 0, 2))

    in_maps = []
    for c in range(N_CORES):
        b0 = c * LC
        in_maps.append({
            "ext": pack(ext_t, b0, H_E),
            "ext64": pack(ext_t, b0 + 64, H_SH),
            "ext32": pack(ext_t, b0 + 32, H_SH),
            "ext96": pack(ext_t, b0 + 96, H_SH),
            "ext8": pack(ext8_t, b0, H_E),
            "ext864": pack(ext8_t, b0 + 64, H_SH),
            "lhs16": lhs16, "lhs8": lhs8,
        })

    trace = bool(os.environ.get("KERNEL_TRACE"))
    if trace:
        _maybe_register_trace_hook()
    res = run_bass_kernel_spmd(nc, in_maps, list(range(N_CORES)), trace=trace)
    LAST_RESULT = res

    out = np.empty((B, 1, NB, L), np.float32)
    for c in range(N_CORES):
        dev = res.results[c]["out"].astype(np.float32)
        for k, si in solo_idx.items():
            mk = plan[k]["mk"]
            arr = dev[si].reshape(128, GROUPS, 4, B)      # [r, g, j, b]
            band = arr.transpose(3, 1, 2, 0).reshape(B, LC) * 2.0 ** (-mk)
            out[:, 0, k, c * LC:(c + 1) * LC] = band
        devp = res.results[c]["outp"].astype(np.float32)
        done = set()
        for k, sec in pair_idx.items():
            if k in done:
                continue
            kb = plan[k].get("partner")
            if plan[k]["kind"] == "pairB":
                k, kb = kb, k
            done.add(k)
            done.add(kb)
            arr = devp[sec].reshape(128, GROUPS, 2, 4, B)  # [r, g, t, j, b]
            a = arr[:64]                                   # [rr, g, t, j, b]
            bb = arr[64:]
            # position = 512g + 128j + 64t + rr
            bandA = a.transpose(4, 1, 3, 2, 0).reshape(B, LC)
            bandB = bb.transpose(4, 1, 3, 2, 0).reshape(B, LC)
            out[:, 0, k, c * LC:(c + 1) * LC] = bandA
            out[:, 0, kb, c * LC:(c + 1) * LC] = bandB
    return out
